# revision 10
# baseline (speedup 1.0000x reference)
"""BandSplitDecoder Trainium2 kernel.

Problem: bands (8, 32, 512, 256) f32; per-band Linear(256 -> 4*w_i) + bias;
scatter into complex64 (8, 2, 1025, 512) as tanh(real) + i*tanh(imag).

Sharding: batch B=8 across the 8 NeuronCores (data parallel, weights
replicated). Per core: x = bands[b] -> output (2, 1025, 512) complex.

Device layout choices:
  - Host pre-transposes bands to (B, K, D, T) so the contraction dim D lands
    on SBUF partitions with no on-chip transposes.
  - Per band i: outT (od_i, 512) = W_i.T @ xT_i computed as accumulating
    matmuls over two K=128 chunks; od_i rows split into <=128-row chunks.
  - Bias + tanh fused in one scalar-engine activation per chunk.
  - Output rows (od order = [re|c0, re|c1, im|c0, im|c1] blocks of w rows)
    DMA'd as contiguous row-runs into separate real/imag DRAM tensors shaped
    (2, 1024, 512); freq bin 1024 is never written by the reference (stays 0).
  - Host assembles complex64 output.
"""

import sys

if "/opt/trn_rl_repo" not in sys.path:
    sys.path.insert(0, "/opt/trn_rl_repo")

import numpy as np

B = 8
K = 32
T = 512
D = 256
C = 2
N_FREQS = 1025
P = 128

# mel band edges for n_bands=32, n_fft=2048, sr=44100 (computed offline from
# the reference's _band_edges; bands exactly tile [0, 1024), bin 1024 unused)
BAND_EDGES = [
    (0, 4), (4, 8), (8, 13), (13, 18), (18, 23), (23, 30), (30, 37),
    (37, 45), (45, 54), (54, 64), (64, 75), (75, 87), (87, 101), (101, 117),
    (117, 134), (134, 153), (153, 174), (174, 198), (198, 224), (224, 254),
    (254, 287), (287, 323), (323, 364), (364, 410), (410, 461), (461, 518),
    (518, 581), (581, 651), (651, 730), (730, 817), (817, 915), (915, 1024),
]
OD_TOTAL = sum(4 * (e - s) for s, e in BAND_EDGES)  # 4096

# Use float32r for the matmuls (full-rate PE at N>=256 vs 4 cycles/row for
# plain fp32). Numerics validated against the f32 reference on hardware.
USE_F32R = True
# Set by test.py for profiling; the grading harness leaves this False.
TRACE = False
LAST_RESULT = None


def _chunk_plan():
    """Split each band's od=4w output rows into <=128-row matmul chunks.

    Returns a list of chunks: dict(band, col0 (global W_cat column), M,
    runs=[(part, c, f0, r0, cnt)]) where rows [r0, r0+cnt) of the chunk map to
    frequency rows [f0, f0+cnt) of channel c in the real (part=0) or imag
    (part=1) output.
    """
    plan = []
    off = 0
    for i, (s, e) in enumerate(BAND_EDGES):
        w = e - s
        od = 4 * w
        o0 = 0
        while o0 < od:
            m = min(P, od - o0)
            runs = []
            r = 0
            while r < m:
                o = o0 + r
                part = o // (2 * w)
                rem = o % (2 * w)
                c = rem // w
                j = rem % w
                cnt = min(m - r, w - j)
                runs.append((part, c, s + j, r, cnt))
                r += cnt
            plan.append(dict(band=i, col0=off + o0, M=m, runs=runs))
            o0 += m
        off += od
    return plan


PLAN = _chunk_plan()
NCHUNK = len(PLAN)  # 51


def _build_bass():
    import concourse.bass as bass
    import concourse.tile as tile
    from concourse import bacc, mybir

    f32 = mybir.dt.float32
    # float32r is 4-byte fp32 data matmul'd at full PE rate with reduced
    # internal precision; np-facing dtype is still float32. The BIR verifier
    # requires the whole producer chain typed f32r, so the x/w DRAM tensors
    # and SBUF tiles are declared f32r directly (bit-identical data).
    fmm = mybir.dt.float32r if USE_F32R else f32
    tanh = mybir.ActivationFunctionType.Tanh

    # Bacc (not raw Bass): its compile() legalizes sync — the ISA has one
    # wait slot per TPB instruction, so multi-wait BIR instructions must be
    # split into event-semaphore waits (generate_event_semaphores) and matmul
    # waits moved onto LDWEIGHTS (move_matmul_waits_to_ldweights).
    nc = bacc.Bacc("TRN2", target_bir_lowering=False, debug=False)
    xT = nc.dram_tensor("xT", [K, D, T], fmm, kind="ExternalInput").ap()
    wd = nc.dram_tensor("w", [D, OD_TOTAL], fmm, kind="ExternalInput").ap()
    biasd = nc.dram_tensor("bias", [P, NCHUNK], f32, kind="ExternalInput").ap()
    yre = nc.dram_tensor("yre", [C, 1024, T], f32, kind="ExternalOutput").ap()
    yim = nc.dram_tensor("yim", [C, 1024, T], f32, kind="ExternalOutput").ap()

    with tile.TileContext(nc) as tc:
        with (
            tc.tile_pool(name="const", bufs=1) as const,
            tc.tile_pool(name="xp", bufs=3) as xp,
            tc.tile_pool(name="op", bufs=4) as op,
            tc.tile_pool(name="pp", bufs=7, space="PSUM") as pp,
        ):
            w_sb = const.tile([P, 2, OD_TOTAL], fmm)
            nc.sync.dma_start(
                out=w_sb[:], in_=wd.rearrange("(kc p) od -> p kc od", p=P)
            )
            bias_sb = const.tile([P, NCHUNK], f32)
            nc.sync.dma_start(out=bias_sb[:], in_=biasd[:])

            # The LDWEIGHTS half of an f32r self-loading matmul accepts only
            # ONE sync wait, so real matmuls must never carry two. A tiny
            # bf16 ldweights "absorber" consumes each freshly-DMA'd tile's
            # semaphore (1 wait, no PSUM write, ~2 cycles); the real matmuls
            # then only ever wait on the PSUM-bank release from the
            # activation engine. The garbage weights it loads are irrelevant:
            # every f32r matmul self-loads its own weights.
            bf16 = mybir.dt.bfloat16
            nc.tensor.ldweights(w_sb[:, 0, 0:1].bitcast(bf16))

            ci = 0
            for i in range(K):
                xt = xp.tile([P, 2, T], fmm, tag="x")
                nc.sync.dma_start(
                    out=xt[:], in_=xT[i].rearrange("(kc p) t -> p kc t", p=P)
                )
                nc.tensor.ldweights(xt[:, 0, 0:1].bitcast(bf16))
                while ci < NCHUNK and PLAN[ci]["band"] == i:
                    ch = PLAN[ci]
                    m, col0 = ch["M"], ch["col0"]
                    ps = pp.tile([P, T], f32, tag="ps")
                    for kc in range(2):
                        lhs = w_sb[:, kc, col0 : col0 + m]
                        rhs = xt[:, kc]
                        nc.tensor.matmul(
                            ps[:m], lhs, rhs, start=(kc == 0), stop=(kc == 1)
                        )
                    ot = op.tile([P, T], f32, tag="ot")
                    nc.scalar.activation(
                        ot[:m], ps[:m], tanh, bias=bias_sb[0:m, ci : ci + 1]
                    )
                    for part, c, f0, r0, cnt in ch["runs"]:
                        dest = yre if part == 0 else yim
                        nc.sync.dma_start(
                            out=dest[c, f0 : f0 + cnt, :], in_=ot[r0 : r0 + cnt, :]
                        )
                    ci += 1
    nc.compile()
    return nc


def kernel(bands, Ws, bs):
    global LAST_RESULT
    from concourse.bass_utils import run_bass_kernel_spmd

    bands = np.asarray(bands, dtype=np.float32)
    Ws = [np.asarray(w, dtype=np.float32) for w in Ws]
    bs = [np.asarray(b, dtype=np.float32) for b in bs]

    xT = np.ascontiguousarray(bands.transpose(0, 1, 3, 2))  # (B, K, D, T)
    w_cat = np.ascontiguousarray(np.concatenate(Ws, axis=1))  # (D, OD_TOTAL)
    b_cat = np.concatenate(bs)
    bias_host = np.zeros((P, NCHUNK), dtype=np.float32)
    for ci, ch in enumerate(PLAN):
        bias_host[: ch["M"], ci] = b_cat[ch["col0"] : ch["col0"] + ch["M"]]

    nc = _build_bass()
    in_maps = [
        {"xT": xT[b], "w": w_cat, "bias": bias_host} for b in range(B)
    ]
    res = run_bass_kernel_spmd(nc, in_maps, list(range(B)), trace=TRACE)
    LAST_RESULT = res

    out = np.zeros((B, C, N_FREQS, T), dtype=np.complex64)
    re = np.stack([res.results[b]["yre"] for b in range(B)])
    im = np.stack([res.results[b]["yim"] for b in range(B)])
    out.real[:, :, :1024, :] = re
    out.imag[:, :, :1024, :] = im
    return out


# revision 13
# speedup vs baseline: 2.1898x; 2.1898x over previous
"""BandSplitDecoder Trainium2 kernel.

Problem: bands (8, 32, 512, 256) f32; per-band Linear(256 -> 4*w_i) + bias;
scatter into complex64 (8, 2, 1025, 512) as tanh(real) + i*tanh(imag).

Sharding: batch B=8 across the 8 NeuronCores (data parallel, weights
replicated). Per core: x = bands[b] -> output (2, 1025, 512) complex.

Device design (v3). The SP sequencer costs ~0.8us per dma_start regardless
of size, so DMA instruction count is minimized and big transfers are used:
  - Host pre-transposes bands to (B, K, D, T) so the contraction dim D lands
    on SBUF partitions with no on-chip transposes.
  - One 1MB input DMA per 2 bands (16 total) + one 4.2MB weight DMA.
  - Per band i: outT (od_i, 512) = W_i.T @ xT_i as accumulating f32r matmuls
    over two K=128 chunks; od_i rows split into <=128-row chunks; bias+tanh
    fused in one scalar-engine activation per chunk (51 chunks total).
  - Each chunk is written with ONE fully-contiguous DMA into y_dev rows
    [chunk_base, chunk_base+M) — y_dev (4096, 512) f32 is simply all chunks
    concatenated (sum od_i = 4096). Out-DMAs alternate between the two HWDGE
    rings (SP and ACT sequencers) to halve issue serialization.
  - The host un-permutes rows into (plane, channel, freq) and assembles
    complex64; freq bin 1024 is never written by the reference (stays 0).
"""

import sys

if "/opt/trn_rl_repo" not in sys.path:
    sys.path.insert(0, "/opt/trn_rl_repo")

import numpy as np

B = 8
K = 32
T = 512
D = 256
C = 2
N_FREQS = 1025
P = 128

# mel band edges for n_bands=32, n_fft=2048, sr=44100 (computed offline from
# the reference's _band_edges; bands exactly tile [0, 1024), bin 1024 unused)
BAND_EDGES = [
    (0, 4), (4, 8), (8, 13), (13, 18), (18, 23), (23, 30), (30, 37),
    (37, 45), (45, 54), (54, 64), (64, 75), (75, 87), (87, 101), (101, 117),
    (117, 134), (134, 153), (153, 174), (174, 198), (198, 224), (224, 254),
    (254, 287), (287, 323), (323, 364), (364, 410), (410, 461), (461, 518),
    (518, 581), (581, 651), (651, 730), (730, 817), (817, 915), (915, 1024),
]
OD_TOTAL = sum(4 * (e - s) for s, e in BAND_EDGES)  # 4096

# float32r matmuls: full-rate PE at N>=256 (plain fp32 is 4 cycles/row).
# Measured on HW: scale-relative max error ~6e-4 vs the f32 reference.
USE_F32R = True
# Set by test.py for profiling; the grading harness leaves this False.
TRACE = False
LAST_RESULT = None


def _chunk_plan():
    """Split each band's od=4w output rows into <=128-row matmul chunks."""
    plan = []
    off = 0
    for i, (s, e) in enumerate(BAND_EDGES):
        od = 4 * (e - s)
        o0 = 0
        while o0 < od:
            m = min(P, od - o0)
            plan.append(dict(band=i, col0=off + o0, M=m))
            o0 += m
        off += od
    return plan


PLAN = _chunk_plan()
NCHUNK = len(PLAN)  # 51


def _row_perm():
    """perm[final_row] = y_dev row, where final rows are plane*2048 +
    c*1024 + f for f in [0, 1024). y_dev rows are chunk-concatenated od
    positions: od index o of band i = part*2*w + c*w + j -> final
    (part, c, s+j)."""
    perm = np.zeros(2 * 2 * 1024, dtype=np.int64)
    dev = 0
    off = 0
    for i, (s, e) in enumerate(BAND_EDGES):
        w = e - s
        for o in range(4 * w):
            part = o // (2 * w)
            c = (o % (2 * w)) // w
            j = o % w
            perm[part * 2048 + c * 1024 + s + j] = off + o
        off += 4 * w
    return perm


ROW_PERM = _row_perm()


def _build_bass():
    import concourse.bass as bass
    import concourse.tile as tile
    from concourse import bacc, mybir

    f32 = mybir.dt.float32
    # float32r is 4-byte fp32 data matmul'd at full PE rate with reduced
    # internal precision; np-facing dtype is still float32. The BIR verifier
    # requires the whole producer chain typed f32r, so the x/w DRAM tensors
    # and SBUF tiles are declared f32r directly (bit-identical data).
    fmm = mybir.dt.float32r if USE_F32R else f32
    bf16 = mybir.dt.bfloat16
    tanh = mybir.ActivationFunctionType.Tanh

    # Bacc (not raw Bass): its compile() legalizes sync — the ISA has one
    # wait slot per TPB instruction, so multi-wait BIR instructions must be
    # split into event-semaphore waits (generate_event_semaphores) and matmul
    # waits moved onto LDWEIGHTS (move_matmul_waits_to_ldweights).
    nc = bacc.Bacc("TRN2", target_bir_lowering=False, debug=False)
    xT = nc.dram_tensor("xT", [K, D, T], fmm, kind="ExternalInput").ap()
    wd = nc.dram_tensor("w", [D, OD_TOTAL], fmm, kind="ExternalInput").ap()
    biasd = nc.dram_tensor("bias", [P, NCHUNK], f32, kind="ExternalInput").ap()
    y = nc.dram_tensor("y", [OD_TOTAL, T], f32, kind="ExternalOutput").ap()

    with tile.TileContext(nc) as tc:
        with (
            tc.tile_pool(name="const", bufs=1) as const,
            tc.tile_pool(name="xp", bufs=3) as xp,
            tc.tile_pool(name="op", bufs=6) as op,
            tc.tile_pool(name="pp", bufs=7, space="PSUM") as pp,
        ):
            w_sb = const.tile([P, 2, OD_TOTAL], fmm)
            nc.sync.dma_start(
                out=w_sb[:], in_=wd.rearrange("(kc p) od -> p kc od", p=P)
            )
            bias_sb = const.tile([P, NCHUNK], f32)
            nc.sync.dma_start(out=bias_sb[:], in_=biasd[:])

            # The LDWEIGHTS half of an f32r self-loading matmul accepts only
            # ONE sync wait, so real matmuls must never carry two. A tiny
            # bf16 ldweights "absorber" consumes each freshly-DMA'd tile's
            # semaphore (1 wait, no PSUM write, ~2 cycles); the real matmuls
            # then only ever wait on the PSUM-bank release from the
            # activation engine. The garbage weights it loads are irrelevant:
            # every f32r matmul self-loads its own weights.
            nc.tensor.ldweights(w_sb[:, 0, 0:1].bitcast(bf16))

            ci = 0
            ring = 0
            for i0 in range(0, K, 2):
                xt = xp.tile([P, 2, 2, T], fmm, tag="x")
                nc.sync.dma_start(
                    out=xt[:],
                    in_=xT[i0 : i0 + 2].rearrange("b (kc p) t -> p b kc t", p=P),
                )
                nc.tensor.ldweights(xt[:, 0, 0, 0:1].bitcast(bf16))
                for i in (i0, i0 + 1):
                    while ci < NCHUNK and PLAN[ci]["band"] == i:
                        ch = PLAN[ci]
                        m, col0 = ch["M"], ch["col0"]
                        ps = pp.tile([P, T], f32, tag="ps")
                        for kc in range(2):
                            nc.tensor.matmul(
                                ps[:m],
                                w_sb[:, kc, col0 : col0 + m],
                                xt[:, i - i0, kc],
                                start=(kc == 0),
                                stop=(kc == 1),
                            )
                        ot = op.tile([P, T], f32, tag="ot")
                        nc.scalar.activation(
                            ot[:m], ps[:m], tanh, bias=bias_sb[0:m, ci : ci + 1]
                        )
                        eng = nc.sync if ring == 0 else nc.scalar
                        ring ^= 1
                        eng.dma_start(
                            out=y[col0 : col0 + m, :], in_=ot[:m, :]
                        )
                        ci += 1
    nc.compile()
    return nc


def kernel(bands, Ws, bs):
    global LAST_RESULT
    from concourse.bass_utils import run_bass_kernel_spmd

    bands = np.asarray(bands, dtype=np.float32)
    Ws = [np.asarray(w, dtype=np.float32) for w in Ws]
    bs = [np.asarray(b, dtype=np.float32) for b in bs]

    xT = np.ascontiguousarray(bands.transpose(0, 1, 3, 2))  # (B, K, D, T)
    w_cat = np.ascontiguousarray(np.concatenate(Ws, axis=1))  # (D, OD_TOTAL)
    b_cat = np.concatenate(bs)
    bias_host = np.zeros((P, NCHUNK), dtype=np.float32)
    for ci, ch in enumerate(PLAN):
        bias_host[: ch["M"], ci] = b_cat[ch["col0"] : ch["col0"] + ch["M"]]

    nc = _build_bass()
    in_maps = [{"xT": xT[b], "w": w_cat, "bias": bias_host} for b in range(B)]
    res = run_bass_kernel_spmd(nc, in_maps, list(range(B)), trace=TRACE)
    LAST_RESULT = res

    out = np.zeros((B, C, N_FREQS, T), dtype=np.complex64)
    yall = np.stack([res.results[b]["y"] for b in range(B)])  # (B, 4096, T)
    g = yall[:, ROW_PERM, :].reshape(B, 2, C, 1024, T)
    out.real[:, :, :1024, :] = g[:, 0]
    out.imag[:, :, :1024, :] = g[:, 1]
    return out


# revision 16
# speedup vs baseline: 2.4360x; 1.1124x over previous
"""BandSplitDecoder Trainium2 kernel.

Problem: bands (8, 32, 512, 256) f32; per-band Linear(256 -> 4*w_i) + bias;
scatter into complex64 (8, 2, 1025, 512) as tanh(real) + i*tanh(imag).

Sharding: batch B=8 across the 8 NeuronCores (data parallel, weights
replicated). Per core: x = bands[b] -> output (2, 1025, 512) complex.

Device design (v3). The SP sequencer costs ~0.8us per dma_start regardless
of size, so DMA instruction count is minimized and big transfers are used:
  - Host pre-transposes bands to (B, K, D, T) so the contraction dim D lands
    on SBUF partitions with no on-chip transposes.
  - One 1MB input DMA per 2 bands (16 total) + one 4.2MB weight DMA.
  - Per band i: outT (od_i, 512) = W_i.T @ xT_i as accumulating f32r matmuls
    over two K=128 chunks; od_i rows split into <=128-row chunks; bias+tanh
    fused in one scalar-engine activation per chunk (51 chunks total).
  - Each chunk is written with ONE fully-contiguous DMA into y_dev rows
    [chunk_base, chunk_base+M) — y_dev (4096, 512) f32 is simply all chunks
    concatenated (sum od_i = 4096). Out-DMAs alternate between the two HWDGE
    rings (SP and ACT sequencers) to halve issue serialization.
  - The host un-permutes rows into (plane, channel, freq) and assembles
    complex64; freq bin 1024 is never written by the reference (stays 0).
"""

import sys

if "/opt/trn_rl_repo" not in sys.path:
    sys.path.insert(0, "/opt/trn_rl_repo")

import numpy as np

B = 8
K = 32
T = 512
D = 256
C = 2
N_FREQS = 1025
P = 128

# mel band edges for n_bands=32, n_fft=2048, sr=44100 (computed offline from
# the reference's _band_edges; bands exactly tile [0, 1024), bin 1024 unused)
BAND_EDGES = [
    (0, 4), (4, 8), (8, 13), (13, 18), (18, 23), (23, 30), (30, 37),
    (37, 45), (45, 54), (54, 64), (64, 75), (75, 87), (87, 101), (101, 117),
    (117, 134), (134, 153), (153, 174), (174, 198), (198, 224), (224, 254),
    (254, 287), (287, 323), (323, 364), (364, 410), (410, 461), (461, 518),
    (518, 581), (581, 651), (651, 730), (730, 817), (817, 915), (915, 1024),
]
OD_TOTAL = sum(4 * (e - s) for s, e in BAND_EDGES)  # 4096

# float32r matmuls: full-rate PE at N>=256 (plain fp32 is 4 cycles/row).
# Measured on HW: scale-relative max error ~6e-4 vs the f32 reference.
USE_F32R = True
# Set by test.py for profiling; the grading harness leaves this False.
TRACE = False
LAST_RESULT = None


def _chunk_plan():
    """Split each band's od=4w output rows into <=128-row matmul chunks."""
    plan = []
    off = 0
    for i, (s, e) in enumerate(BAND_EDGES):
        od = 4 * (e - s)
        o0 = 0
        while o0 < od:
            m = min(P, od - o0)
            plan.append(dict(band=i, col0=off + o0, M=m))
            o0 += m
        off += od
    return plan


PLAN = _chunk_plan()
NCHUNK = len(PLAN)  # 51


def _row_perm():
    """perm[final_row] = y_dev row, where final rows are plane*2048 +
    c*1024 + f for f in [0, 1024). y_dev rows are chunk-concatenated od
    positions: od index o of band i = part*2*w + c*w + j -> final
    (part, c, s+j)."""
    perm = np.zeros(2 * 2 * 1024, dtype=np.int64)
    dev = 0
    off = 0
    for i, (s, e) in enumerate(BAND_EDGES):
        w = e - s
        for o in range(4 * w):
            part = o // (2 * w)
            c = (o % (2 * w)) // w
            j = o % w
            perm[part * 2048 + c * 1024 + s + j] = off + o
        off += 4 * w
    return perm


ROW_PERM = _row_perm()


def _build_bass():
    import concourse.bass as bass
    import concourse.tile as tile
    from concourse import bacc, mybir

    f32 = mybir.dt.float32
    # float32r is 4-byte fp32 data matmul'd at full PE rate with reduced
    # internal precision; np-facing dtype is still float32. The BIR verifier
    # requires the whole producer chain typed f32r, so the x/w DRAM tensors
    # and SBUF tiles are declared f32r directly (bit-identical data).
    fmm = mybir.dt.float32r if USE_F32R else f32
    bf16 = mybir.dt.bfloat16
    tanh = mybir.ActivationFunctionType.Tanh

    # Bacc (not raw Bass): its compile() legalizes sync — the ISA has one
    # wait slot per TPB instruction, so multi-wait BIR instructions must be
    # split into event-semaphore waits (generate_event_semaphores) and matmul
    # waits moved onto LDWEIGHTS (move_matmul_waits_to_ldweights).
    nc = bacc.Bacc("TRN2", target_bir_lowering=False, debug=False)
    xT = nc.dram_tensor("xT", [K, D, T], fmm, kind="ExternalInput").ap()
    wd = nc.dram_tensor("w", [D, OD_TOTAL], fmm, kind="ExternalInput").ap()
    biasd = nc.dram_tensor("bias", [P, NCHUNK], f32, kind="ExternalInput").ap()
    y = nc.dram_tensor("y", [OD_TOTAL, T], f32, kind="ExternalOutput").ap()

    with tile.TileContext(nc) as tc:
        with (
            tc.tile_pool(name="const", bufs=1) as const,
            tc.tile_pool(name="xp", bufs=4) as xp,
            tc.tile_pool(name="op", bufs=6) as op,
            tc.tile_pool(name="pp", bufs=7, space="PSUM") as pp,
        ):
            # two separate weight tiles: the kc=0 matmuls only wait on the
            # first (half-sized) load, halving compute start latency
            w_sb = [const.tile([P, OD_TOTAL], fmm, name=f"w{kc}") for kc in (0, 1)]
            nc.sync.dma_start(out=w_sb[0][:], in_=wd[0:P, :])
            nc.sync.dma_start(out=w_sb[1][:], in_=wd[P : 2 * P, :])
            bias_sb = const.tile([P, NCHUNK], f32)
            nc.sync.dma_start(out=bias_sb[:], in_=biasd[:])

            # The LDWEIGHTS half of an f32r self-loading matmul accepts only
            # ONE sync wait, so real matmuls must never carry two. A tiny
            # bf16 ldweights "absorber" consumes each freshly-DMA'd tile's
            # semaphore (1 wait, no PSUM write, ~2 cycles); the real matmuls
            # then only ever wait on the PSUM-bank release from the
            # activation engine. The garbage weights it loads are irrelevant:
            # every f32r matmul self-loads its own weights.
            nc.tensor.ldweights(w_sb[0][:, 0:1].bitcast(bf16))
            nc.tensor.ldweights(w_sb[1][:, 0:1].bitcast(bf16))

            ci = 0
            ring = 0
            for i0 in range(0, K, 2):
                xt = xp.tile([P, 2, 2, T], fmm, tag="x")
                # input loads go through SWDGE (gpsimd) so their transfers
                # ride a separate DMA queue from the HWDGE output rings
                nc.gpsimd.dma_start(
                    out=xt[:],
                    in_=xT[i0 : i0 + 2].rearrange("b (kc p) t -> p b kc t", p=P),
                )
                nc.tensor.ldweights(xt[:, 0, 0, 0:1].bitcast(bf16))
                for i in (i0, i0 + 1):
                    while ci < NCHUNK and PLAN[ci]["band"] == i:
                        ch = PLAN[ci]
                        m, col0 = ch["M"], ch["col0"]
                        ps = pp.tile([P, T], f32, tag="ps")
                        for kc in range(2):
                            nc.tensor.matmul(
                                ps[:m],
                                w_sb[kc][:, col0 : col0 + m],
                                xt[:, i - i0, kc],
                                start=(kc == 0),
                                stop=(kc == 1),
                            )
                        ot = op.tile([P, T], f32, tag="ot")
                        nc.scalar.activation(
                            ot[:m], ps[:m], tanh, bias=bias_sb[0:m, ci : ci + 1]
                        )
                        eng = nc.sync if ring == 0 else nc.scalar
                        ring ^= 1
                        eng.dma_start(
                            out=y[col0 : col0 + m, :], in_=ot[:m, :]
                        )
                        ci += 1
    nc.compile()
    return nc


def kernel(bands, Ws, bs):
    global LAST_RESULT
    from concourse.bass_utils import run_bass_kernel_spmd

    bands = np.asarray(bands, dtype=np.float32)
    Ws = [np.asarray(w, dtype=np.float32) for w in Ws]
    bs = [np.asarray(b, dtype=np.float32) for b in bs]

    xT = np.ascontiguousarray(bands.transpose(0, 1, 3, 2))  # (B, K, D, T)
    w_cat = np.ascontiguousarray(np.concatenate(Ws, axis=1))  # (D, OD_TOTAL)
    b_cat = np.concatenate(bs)
    bias_host = np.zeros((P, NCHUNK), dtype=np.float32)
    for ci, ch in enumerate(PLAN):
        bias_host[: ch["M"], ci] = b_cat[ch["col0"] : ch["col0"] + ch["M"]]

    nc = _build_bass()
    in_maps = [{"xT": xT[b], "w": w_cat, "bias": bias_host} for b in range(B)]
    res = run_bass_kernel_spmd(nc, in_maps, list(range(B)), trace=TRACE)
    LAST_RESULT = res

    out = np.zeros((B, C, N_FREQS, T), dtype=np.complex64)
    yall = np.stack([res.results[b]["y"] for b in range(B)])  # (B, 4096, T)
    g = yall[:, ROW_PERM, :].reshape(B, 2, C, 1024, T)
    out.real[:, :, :1024, :] = g[:, 0]
    out.imag[:, :, :1024, :] = g[:, 1]
    return out


# revision 21
# speedup vs baseline: 2.5454x; 1.0449x over previous
"""BandSplitDecoder Trainium2 kernel.

Problem: bands (8, 32, 512, 256) f32; per-band Linear(256 -> 4*w_i) + bias;
scatter into complex64 (8, 2, 1025, 512) as tanh(real) + i*tanh(imag).

Sharding: batch B=8 across the 8 NeuronCores (data parallel, weights
replicated). Per core: x = bands[b] -> output (2, 1025, 512) complex.

Device design (v3). The SP sequencer costs ~0.8us per dma_start regardless
of size, so DMA instruction count is minimized and big transfers are used:
  - Host pre-transposes bands to (B, K, D, T) so the contraction dim D lands
    on SBUF partitions with no on-chip transposes.
  - One 1MB input DMA per 2 bands (16 total) + one 4.2MB weight DMA.
  - Per band i: outT (od_i, 512) = W_i.T @ xT_i as accumulating f32r matmuls
    over two K=128 chunks; od_i rows split into <=128-row chunks; bias+tanh
    fused in one scalar-engine activation per chunk (51 chunks total).
  - Each chunk is written with ONE fully-contiguous DMA into y_dev rows
    [chunk_base, chunk_base+M) — y_dev (4096, 512) f32 is simply all chunks
    concatenated (sum od_i = 4096). Out-DMAs alternate between the two HWDGE
    rings (SP and ACT sequencers) to halve issue serialization.
  - The host un-permutes rows into (plane, channel, freq) and assembles
    complex64; freq bin 1024 is never written by the reference (stays 0).
"""

import sys

if "/opt/trn_rl_repo" not in sys.path:
    sys.path.insert(0, "/opt/trn_rl_repo")

import numpy as np

B = 8
K = 32
T = 512
D = 256
C = 2
N_FREQS = 1025
P = 128

# mel band edges for n_bands=32, n_fft=2048, sr=44100 (computed offline from
# the reference's _band_edges; bands exactly tile [0, 1024), bin 1024 unused)
BAND_EDGES = [
    (0, 4), (4, 8), (8, 13), (13, 18), (18, 23), (23, 30), (30, 37),
    (37, 45), (45, 54), (54, 64), (64, 75), (75, 87), (87, 101), (101, 117),
    (117, 134), (134, 153), (153, 174), (174, 198), (198, 224), (224, 254),
    (254, 287), (287, 323), (323, 364), (364, 410), (410, 461), (461, 518),
    (518, 581), (581, 651), (651, 730), (730, 817), (817, 915), (915, 1024),
]
OD_TOTAL = sum(4 * (e - s) for s, e in BAND_EDGES)  # 4096

# float32r matmuls: full-rate PE at N>=256 (plain fp32 is 4 cycles/row).
# Measured on HW: scale-relative max error ~6e-4 vs the f32 reference.
USE_F32R = True
# Set by test.py for profiling; the grading harness leaves this False.
TRACE = False
LAST_RESULT = None


def _chunk_plan():
    """Split each band's od=4w output rows into <=128-row matmul chunks."""
    plan = []
    off = 0
    for i, (s, e) in enumerate(BAND_EDGES):
        od = 4 * (e - s)
        o0 = 0
        while o0 < od:
            m = min(P, od - o0)
            plan.append(dict(band=i, col0=off + o0, M=m))
            o0 += m
        off += od
    return plan


PLAN = _chunk_plan()
NCHUNK = len(PLAN)  # 51


def _row_perm():
    """perm[final_row] = y_dev row, where final rows are plane*2048 +
    c*1024 + f for f in [0, 1024). y_dev rows are chunk-concatenated od
    positions: od index o of band i = part*2*w + c*w + j -> final
    (part, c, s+j)."""
    perm = np.zeros(2 * 2 * 1024, dtype=np.int64)
    dev = 0
    off = 0
    for i, (s, e) in enumerate(BAND_EDGES):
        w = e - s
        for o in range(4 * w):
            part = o // (2 * w)
            c = (o % (2 * w)) // w
            j = o % w
            perm[part * 2048 + c * 1024 + s + j] = off + o
        off += 4 * w
    return perm


ROW_PERM = _row_perm()


def _build_bass():
    import concourse.bass as bass
    import concourse.tile as tile
    from concourse import bacc, mybir

    f32 = mybir.dt.float32
    # float32r is 4-byte fp32 data matmul'd at full PE rate with reduced
    # internal precision; np-facing dtype is still float32. The BIR verifier
    # requires the whole producer chain typed f32r, so the x/w DRAM tensors
    # and SBUF tiles are declared f32r directly (bit-identical data).
    fmm = mybir.dt.float32r if USE_F32R else f32
    bf16 = mybir.dt.bfloat16
    tanh = mybir.ActivationFunctionType.Tanh

    # Bacc (not raw Bass): its compile() legalizes sync — the ISA has one
    # wait slot per TPB instruction, so multi-wait BIR instructions must be
    # split into event-semaphore waits (generate_event_semaphores) and matmul
    # waits moved onto LDWEIGHTS (move_matmul_waits_to_ldweights).
    # 4 SWDGE queues so the 16 input DMAs pipeline without Q7 drain stalls
    nc = bacc.Bacc(
        "TRN2",
        target_bir_lowering=False,
        debug=False,
        num_swdge_queues=4,
    )
    xT = nc.dram_tensor("xT", [K, D, T], fmm, kind="ExternalInput").ap()
    wd = nc.dram_tensor("w", [D, OD_TOTAL], fmm, kind="ExternalInput").ap()
    biasd = nc.dram_tensor("bias", [P, NCHUNK], f32, kind="ExternalInput").ap()
    y = nc.dram_tensor("y", [OD_TOTAL, T], f32, kind="ExternalOutput").ap()

    with tile.TileContext(nc) as tc:
        with (
            tc.tile_pool(name="const", bufs=1) as const,
            tc.tile_pool(name="xp", bufs=4) as xp,
            tc.tile_pool(name="op", bufs=6) as op,
            tc.tile_pool(name="pp", bufs=7, space="PSUM") as pp,
        ):
            # weight tiles split by k-chunk AND column half (split point 1972
            # is a chunk boundary): the first bands' matmuls gate on a 1MB
            # load instead of the whole 4.2MB weight matrix
            WSPLIT = 1972
            w_sb = [
                [
                    const.tile([P, WSPLIT], fmm, name=f"w{kc}l", tag=f"w{kc}l"),
                    const.tile([P, OD_TOTAL - WSPLIT], fmm, name=f"w{kc}r", tag=f"w{kc}r"),
                ]
                for kc in (0, 1)
            ]
            for kc in (0, 1):
                nc.sync.dma_start(
                    out=w_sb[kc][0][:], in_=wd[kc * P : kc * P + P, :WSPLIT]
                )
                nc.sync.dma_start(
                    out=w_sb[kc][1][:], in_=wd[kc * P : kc * P + P, WSPLIT:]
                )
            bias_sb = const.tile([P, NCHUNK], f32)
            nc.sync.dma_start(out=bias_sb[:], in_=biasd[:])

            # The LDWEIGHTS half of an f32r self-loading matmul accepts only
            # ONE sync wait, so real matmuls must never carry two. A tiny
            # bf16 ldweights "absorber" consumes each freshly-DMA'd tile's
            # semaphore (1 wait, no PSUM write, ~2 cycles); the real matmuls
            # then only ever wait on the PSUM-bank release from the
            # activation engine. The garbage weights it loads are irrelevant:
            # every f32r matmul self-loads its own weights.
            for kc in (0, 1):
                for half in (0, 1):
                    nc.tensor.ldweights(w_sb[kc][half][:, 0:1].bitcast(bf16))

            ci = 0
            ring = 0
            for i0 in range(0, K, 2):
                xt = xp.tile([P, 2, 2, T], fmm, tag="x")
                # input loads go through SWDGE (gpsimd) so their transfers
                # ride a separate DMA queue from the HWDGE output rings
                nc.gpsimd.dma_start(
                    out=xt[:],
                    in_=xT[i0 : i0 + 2].rearrange("b (kc p) t -> p b kc t", p=P),
                )
                nc.tensor.ldweights(xt[:, 0, 0, 0:1].bitcast(bf16))
                for i in (i0, i0 + 1):
                    while ci < NCHUNK and PLAN[ci]["band"] == i:
                        ch = PLAN[ci]
                        m, col0 = ch["M"], ch["col0"]
                        ps = pp.tile([P, T], f32, tag="ps")
                        for kc in range(2):
                            if col0 < WSPLIT:
                                lhs = w_sb[kc][0][:, col0 : col0 + m]
                            else:
                                lhs = w_sb[kc][1][:, col0 - WSPLIT : col0 - WSPLIT + m]
                            nc.tensor.matmul(
                                ps[:m],
                                lhs,
                                xt[:, i - i0, kc],
                                start=(kc == 0),
                                stop=(kc == 1),
                            )
                        ot = op.tile([P, T], f32, tag="ot")
                        nc.scalar.activation(
                            ot[:m], ps[:m], tanh, bias=bias_sb[0:m, ci : ci + 1]
                        )
                        eng = nc.sync if ring == 0 else nc.scalar
                        ring ^= 1
                        eng.dma_start(
                            out=y[col0 : col0 + m, :], in_=ot[:m, :]
                        )
                        ci += 1
    nc.compile()
    return nc


def kernel(bands, Ws, bs):
    global LAST_RESULT
    from concourse.bass_utils import run_bass_kernel_spmd

    bands = np.asarray(bands, dtype=np.float32)
    Ws = [np.asarray(w, dtype=np.float32) for w in Ws]
    bs = [np.asarray(b, dtype=np.float32) for b in bs]

    xT = np.ascontiguousarray(bands.transpose(0, 1, 3, 2))  # (B, K, D, T)
    w_cat = np.ascontiguousarray(np.concatenate(Ws, axis=1))  # (D, OD_TOTAL)
    b_cat = np.concatenate(bs)
    bias_host = np.zeros((P, NCHUNK), dtype=np.float32)
    for ci, ch in enumerate(PLAN):
        bias_host[: ch["M"], ci] = b_cat[ch["col0"] : ch["col0"] + ch["M"]]

    nc = _build_bass()
    in_maps = [{"xT": xT[b], "w": w_cat, "bias": bias_host} for b in range(B)]
    res = run_bass_kernel_spmd(nc, in_maps, list(range(B)), trace=TRACE)
    LAST_RESULT = res

    out = np.zeros((B, C, N_FREQS, T), dtype=np.complex64)
    yall = np.stack([res.results[b]["y"] for b in range(B)])  # (B, 4096, T)
    g = yall[:, ROW_PERM, :].reshape(B, 2, C, 1024, T)
    out.real[:, :, :1024, :] = g[:, 0]
    out.imag[:, :, :1024, :] = g[:, 1]
    return out


# revision 23
# speedup vs baseline: 2.5649x; 1.0077x over previous
"""BandSplitDecoder Trainium2 kernel.

Problem: bands (8, 32, 512, 256) f32; per-band Linear(256 -> 4*w_i) + bias;
scatter into complex64 (8, 2, 1025, 512) as tanh(real) + i*tanh(imag).

Sharding: batch B=8 across the 8 NeuronCores (data parallel, weights
replicated). Per core: x = bands[b] -> output (2, 1025, 512) complex.

Device design (v3). The SP sequencer costs ~0.8us per dma_start regardless
of size, so DMA instruction count is minimized and big transfers are used:
  - Host pre-transposes bands to (B, K, D, T) so the contraction dim D lands
    on SBUF partitions with no on-chip transposes.
  - One 1MB input DMA per 2 bands (16 total) + one 4.2MB weight DMA.
  - Per band i: outT (od_i, 512) = W_i.T @ xT_i as accumulating f32r matmuls
    over two K=128 chunks; od_i rows split into <=128-row chunks; bias+tanh
    fused in one scalar-engine activation per chunk (51 chunks total).
  - Each chunk is written with ONE fully-contiguous DMA into y_dev rows
    [chunk_base, chunk_base+M) — y_dev (4096, 512) f32 is simply all chunks
    concatenated (sum od_i = 4096). Out-DMAs alternate between the two HWDGE
    rings (SP and ACT sequencers) to halve issue serialization.
  - The host un-permutes rows into (plane, channel, freq) and assembles
    complex64; freq bin 1024 is never written by the reference (stays 0).
"""

import sys

if "/opt/trn_rl_repo" not in sys.path:
    sys.path.insert(0, "/opt/trn_rl_repo")

import numpy as np

B = 8
K = 32
T = 512
D = 256
C = 2
N_FREQS = 1025
P = 128

# mel band edges for n_bands=32, n_fft=2048, sr=44100 (computed offline from
# the reference's _band_edges; bands exactly tile [0, 1024), bin 1024 unused)
BAND_EDGES = [
    (0, 4), (4, 8), (8, 13), (13, 18), (18, 23), (23, 30), (30, 37),
    (37, 45), (45, 54), (54, 64), (64, 75), (75, 87), (87, 101), (101, 117),
    (117, 134), (134, 153), (153, 174), (174, 198), (198, 224), (224, 254),
    (254, 287), (287, 323), (323, 364), (364, 410), (410, 461), (461, 518),
    (518, 581), (581, 651), (651, 730), (730, 817), (817, 915), (915, 1024),
]
OD_TOTAL = sum(4 * (e - s) for s, e in BAND_EDGES)  # 4096

# float32r matmuls: full-rate PE at N>=256 (plain fp32 is 4 cycles/row).
# Measured on HW: scale-relative max error ~6e-4 vs the f32 reference.
USE_F32R = True
# Set by test.py for profiling; the grading harness leaves this False.
TRACE = False
LAST_RESULT = None


def _chunk_plan():
    """Split each band's od=4w output rows into <=128-row matmul chunks."""
    plan = []
    off = 0
    for i, (s, e) in enumerate(BAND_EDGES):
        od = 4 * (e - s)
        o0 = 0
        while o0 < od:
            m = min(P, od - o0)
            plan.append(dict(band=i, col0=off + o0, M=m))
            o0 += m
        off += od
    return plan


PLAN = _chunk_plan()
NCHUNK = len(PLAN)  # 51


def _row_perm():
    """perm[final_row] = y_dev row, where final rows are plane*2048 +
    c*1024 + f for f in [0, 1024). y_dev rows are chunk-concatenated od
    positions: od index o of band i = part*2*w + c*w + j -> final
    (part, c, s+j)."""
    perm = np.zeros(2 * 2 * 1024, dtype=np.int64)
    dev = 0
    off = 0
    for i, (s, e) in enumerate(BAND_EDGES):
        w = e - s
        for o in range(4 * w):
            part = o // (2 * w)
            c = (o % (2 * w)) // w
            j = o % w
            perm[part * 2048 + c * 1024 + s + j] = off + o
        off += 4 * w
    return perm


ROW_PERM = _row_perm()


def _build_bass():
    import concourse.bass as bass
    import concourse.tile as tile
    from concourse import bacc, mybir

    f32 = mybir.dt.float32
    # float32r is 4-byte fp32 data matmul'd at full PE rate with reduced
    # internal precision; np-facing dtype is still float32. The BIR verifier
    # requires the whole producer chain typed f32r, so the x/w DRAM tensors
    # and SBUF tiles are declared f32r directly (bit-identical data).
    fmm = mybir.dt.float32r if USE_F32R else f32
    bf16 = mybir.dt.bfloat16
    tanh = mybir.ActivationFunctionType.Tanh

    # Bacc (not raw Bass): its compile() legalizes sync — the ISA has one
    # wait slot per TPB instruction, so multi-wait BIR instructions must be
    # split into event-semaphore waits (generate_event_semaphores) and matmul
    # waits moved onto LDWEIGHTS (move_matmul_waits_to_ldweights).
    # 4 SWDGE queues so the 16 input DMAs pipeline without Q7 drain stalls
    nc = bacc.Bacc(
        "TRN2",
        target_bir_lowering=False,
        debug=False,
        num_swdge_queues=4,
    )
    xT = nc.dram_tensor("xT", [K, D, T], fmm, kind="ExternalInput").ap()
    wd = nc.dram_tensor("w", [D, OD_TOTAL], fmm, kind="ExternalInput").ap()
    biasd = nc.dram_tensor("bias", [P, NCHUNK], f32, kind="ExternalInput").ap()
    y = nc.dram_tensor("y", [OD_TOTAL, T], f32, kind="ExternalOutput").ap()

    with tile.TileContext(nc) as tc:
        with (
            tc.tile_pool(name="const", bufs=1) as const,
            tc.tile_pool(name="xp", bufs=4) as xp,
            tc.tile_pool(name="op", bufs=6) as op,
            tc.tile_pool(name="pp", bufs=7, space="PSUM") as pp,
        ):
            # weight tiles split by k-chunk AND column half (split point 1972
            # is a chunk boundary): the first bands' matmuls gate on a 1MB
            # load instead of the whole 4.2MB weight matrix
            WSPLIT = 1972
            w_sb = [
                [
                    const.tile([P, WSPLIT], fmm, name=f"w{kc}l", tag=f"w{kc}l"),
                    const.tile([P, OD_TOTAL - WSPLIT], fmm, name=f"w{kc}r", tag=f"w{kc}r"),
                ]
                for kc in (0, 1)
            ]
            for kc in (0, 1):
                nc.sync.dma_start(
                    out=w_sb[kc][0][:], in_=wd[kc * P : kc * P + P, :WSPLIT]
                )
                nc.sync.dma_start(
                    out=w_sb[kc][1][:], in_=wd[kc * P : kc * P + P, WSPLIT:]
                )
            bias_sb = const.tile([P, NCHUNK], f32)
            nc.sync.dma_start(out=bias_sb[:], in_=biasd[:])

            # The LDWEIGHTS half of an f32r self-loading matmul accepts only
            # ONE sync wait, so real matmuls must never carry two. A tiny
            # bf16 ldweights "absorber" consumes each freshly-DMA'd tile's
            # semaphore (1 wait, no PSUM write, ~2 cycles); the real matmuls
            # then only ever wait on the PSUM-bank release from the
            # activation engine. The garbage weights it loads are irrelevant:
            # every f32r matmul self-loads its own weights.
            # absorb only the LEFT weight halves now; the right halves are
            # absorbed immediately before their first use (band 25) so the
            # in-order PE doesn't stall on the full 4.2MB weight load
            nc.tensor.ldweights(w_sb[0][0][:, 0:1].bitcast(bf16))
            nc.tensor.ldweights(w_sb[1][0][:, 0:1].bitcast(bf16))
            w_right_absorbed = False

            ci = 0
            ring = 0
            for i0 in range(0, K, 2):
                xt = xp.tile([P, 2, 2, T], fmm, tag="x")
                # input loads go through SWDGE (gpsimd) so their transfers
                # ride a separate DMA queue from the HWDGE output rings
                nc.gpsimd.dma_start(
                    out=xt[:],
                    in_=xT[i0 : i0 + 2].rearrange("b (kc p) t -> p b kc t", p=P),
                )
                nc.tensor.ldweights(xt[:, 0, 0, 0:1].bitcast(bf16))
                for i in (i0, i0 + 1):
                    while ci < NCHUNK and PLAN[ci]["band"] == i:
                        ch = PLAN[ci]
                        m, col0 = ch["M"], ch["col0"]
                        if col0 >= WSPLIT and not w_right_absorbed:
                            nc.tensor.ldweights(w_sb[0][1][:, 0:1].bitcast(bf16))
                            nc.tensor.ldweights(w_sb[1][1][:, 0:1].bitcast(bf16))
                            w_right_absorbed = True
                        ps = pp.tile([P, T], f32, tag="ps")
                        for kc in range(2):
                            if col0 < WSPLIT:
                                lhs = w_sb[kc][0][:, col0 : col0 + m]
                            else:
                                lhs = w_sb[kc][1][:, col0 - WSPLIT : col0 - WSPLIT + m]
                            nc.tensor.matmul(
                                ps[:m],
                                lhs,
                                xt[:, i - i0, kc],
                                start=(kc == 0),
                                stop=(kc == 1),
                            )
                        ot = op.tile([P, T], f32, tag="ot")
                        nc.scalar.activation(
                            ot[:m], ps[:m], tanh, bias=bias_sb[0:m, ci : ci + 1]
                        )
                        eng = nc.sync if ring == 0 else nc.scalar
                        ring ^= 1
                        eng.dma_start(
                            out=y[col0 : col0 + m, :], in_=ot[:m, :]
                        )
                        ci += 1
    nc.compile()
    return nc


def kernel(bands, Ws, bs):
    global LAST_RESULT
    from concourse.bass_utils import run_bass_kernel_spmd

    bands = np.asarray(bands, dtype=np.float32)
    Ws = [np.asarray(w, dtype=np.float32) for w in Ws]
    bs = [np.asarray(b, dtype=np.float32) for b in bs]

    xT = np.ascontiguousarray(bands.transpose(0, 1, 3, 2))  # (B, K, D, T)
    w_cat = np.ascontiguousarray(np.concatenate(Ws, axis=1))  # (D, OD_TOTAL)
    b_cat = np.concatenate(bs)
    bias_host = np.zeros((P, NCHUNK), dtype=np.float32)
    for ci, ch in enumerate(PLAN):
        bias_host[: ch["M"], ci] = b_cat[ch["col0"] : ch["col0"] + ch["M"]]

    nc = _build_bass()
    in_maps = [{"xT": xT[b], "w": w_cat, "bias": bias_host} for b in range(B)]
    res = run_bass_kernel_spmd(nc, in_maps, list(range(B)), trace=TRACE)
    LAST_RESULT = res

    out = np.zeros((B, C, N_FREQS, T), dtype=np.complex64)
    yall = np.stack([res.results[b]["y"] for b in range(B)])  # (B, 4096, T)
    g = yall[:, ROW_PERM, :].reshape(B, 2, C, 1024, T)
    out.real[:, :, :1024, :] = g[:, 0]
    out.imag[:, :, :1024, :] = g[:, 1]
    return out


# revision 26
# speedup vs baseline: 2.5932x; 1.0110x over previous
"""BandSplitDecoder Trainium2 kernel.

Problem: bands (8, 32, 512, 256) f32; per-band Linear(256 -> 4*w_i) + bias;
scatter into complex64 (8, 2, 1025, 512) as tanh(real) + i*tanh(imag).

Sharding: batch B=8 across the 8 NeuronCores (data parallel, weights
replicated). Per core: x = bands[b] -> output (2, 1025, 512) complex.

Device design (v3). The SP sequencer costs ~0.8us per dma_start regardless
of size, so DMA instruction count is minimized and big transfers are used:
  - Host pre-transposes bands to (B, K, D, T) so the contraction dim D lands
    on SBUF partitions with no on-chip transposes.
  - One 1MB input DMA per 2 bands (16 total) + one 4.2MB weight DMA.
  - Per band i: outT (od_i, 512) = W_i.T @ xT_i as accumulating f32r matmuls
    over two K=128 chunks; od_i rows split into <=128-row chunks; bias+tanh
    fused in one scalar-engine activation per chunk (51 chunks total).
  - Each chunk is written with ONE fully-contiguous DMA into y_dev rows
    [chunk_base, chunk_base+M) — y_dev (4096, 512) f32 is simply all chunks
    concatenated (sum od_i = 4096). Out-DMAs alternate between the two HWDGE
    rings (SP and ACT sequencers) to halve issue serialization.
  - The host un-permutes rows into (plane, channel, freq) and assembles
    complex64; freq bin 1024 is never written by the reference (stays 0).
"""

import sys

if "/opt/trn_rl_repo" not in sys.path:
    sys.path.insert(0, "/opt/trn_rl_repo")

import numpy as np

B = 8
K = 32
T = 512
D = 256
C = 2
N_FREQS = 1025
P = 128

# mel band edges for n_bands=32, n_fft=2048, sr=44100 (computed offline from
# the reference's _band_edges; bands exactly tile [0, 1024), bin 1024 unused)
BAND_EDGES = [
    (0, 4), (4, 8), (8, 13), (13, 18), (18, 23), (23, 30), (30, 37),
    (37, 45), (45, 54), (54, 64), (64, 75), (75, 87), (87, 101), (101, 117),
    (117, 134), (134, 153), (153, 174), (174, 198), (198, 224), (224, 254),
    (254, 287), (287, 323), (323, 364), (364, 410), (410, 461), (461, 518),
    (518, 581), (581, 651), (651, 730), (730, 817), (817, 915), (915, 1024),
]
OD_TOTAL = sum(4 * (e - s) for s, e in BAND_EDGES)  # 4096

# float32r matmuls: full-rate PE at N>=256 (plain fp32 is 4 cycles/row).
# Measured on HW: scale-relative max error ~6e-4 vs the f32 reference.
USE_F32R = True
# Ship x and W as bf16 (halves input HBM traffic; ~3e-3 scale-relative error)
MM_BF16 = False
# Set by test.py for profiling; the grading harness leaves this False.
TRACE = False
LAST_RESULT = None


def _chunk_plan():
    """Split each band's od=4w output rows into <=128-row matmul chunks."""
    plan = []
    off = 0
    for i, (s, e) in enumerate(BAND_EDGES):
        od = 4 * (e - s)
        o0 = 0
        while o0 < od:
            m = min(P, od - o0)
            plan.append(dict(band=i, col0=off + o0, M=m))
            o0 += m
        off += od
    return plan


PLAN = _chunk_plan()
NCHUNK = len(PLAN)  # 51


def _row_perm():
    """perm[final_row] = y_dev row, where final rows are plane*2048 +
    c*1024 + f for f in [0, 1024). y_dev rows are chunk-concatenated od
    positions: od index o of band i = part*2*w + c*w + j -> final
    (part, c, s+j)."""
    perm = np.zeros(2 * 2 * 1024, dtype=np.int64)
    dev = 0
    off = 0
    for i, (s, e) in enumerate(BAND_EDGES):
        w = e - s
        for o in range(4 * w):
            part = o // (2 * w)
            c = (o % (2 * w)) // w
            j = o % w
            perm[part * 2048 + c * 1024 + s + j] = off + o
        off += 4 * w
    return perm


ROW_PERM = _row_perm()


def _build_bass():
    import concourse.bass as bass
    import concourse.tile as tile
    from concourse import bacc, mybir

    f32 = mybir.dt.float32
    # float32r is 4-byte fp32 data matmul'd at full PE rate with reduced
    # internal precision; np-facing dtype is still float32. The BIR verifier
    # requires the whole producer chain typed f32r, so the x/w DRAM tensors
    # and SBUF tiles are declared f32r directly (bit-identical data).
    bf16 = mybir.dt.bfloat16
    if MM_BF16:
        fmm = bf16
    else:
        fmm = mybir.dt.float32r if USE_F32R else f32
    tanh = mybir.ActivationFunctionType.Tanh

    # Bacc (not raw Bass): its compile() legalizes sync — the ISA has one
    # wait slot per TPB instruction, so multi-wait BIR instructions must be
    # split into event-semaphore waits (generate_event_semaphores) and matmul
    # waits moved onto LDWEIGHTS (move_matmul_waits_to_ldweights).
    # 4 SWDGE queues so the 16 input DMAs pipeline without Q7 drain stalls
    nc = bacc.Bacc(
        "TRN2",
        target_bir_lowering=False,
        debug=False,
        num_swdge_queues=4,
    )
    xT = nc.dram_tensor("xT", [K, D, T], fmm, kind="ExternalInput").ap()
    wd = nc.dram_tensor("w", [D, OD_TOTAL], fmm, kind="ExternalInput").ap()
    biasd = nc.dram_tensor("bias", [P, NCHUNK], f32, kind="ExternalInput").ap()
    y = nc.dram_tensor("y", [OD_TOTAL, T], f32, kind="ExternalOutput").ap()

    with tile.TileContext(nc) as tc:
        with (
            tc.tile_pool(name="const", bufs=1) as const,
            tc.tile_pool(name="xp", bufs=4) as xp,
            tc.tile_pool(name="op", bufs=6) as op,
            tc.tile_pool(name="pp", bufs=7, space="PSUM") as pp,
        ):
            # weight tiles split by k-chunk AND column half (split point 1972
            # is a chunk boundary): the first bands' matmuls gate on a 1MB
            # load instead of the whole 4.2MB weight matrix
            WSPLIT = 1972
            w_sb = [
                [
                    const.tile([P, WSPLIT], fmm, name=f"w{kc}l", tag=f"w{kc}l"),
                    const.tile([P, OD_TOTAL - WSPLIT], fmm, name=f"w{kc}r", tag=f"w{kc}r"),
                ]
                for kc in (0, 1)
            ]
            for kc in (0, 1):
                nc.sync.dma_start(
                    out=w_sb[kc][0][:], in_=wd[kc * P : kc * P + P, :WSPLIT]
                )
                nc.sync.dma_start(
                    out=w_sb[kc][1][:], in_=wd[kc * P : kc * P + P, WSPLIT:]
                )
            bias_sb = const.tile([P, NCHUNK], f32)
            nc.sync.dma_start(out=bias_sb[:], in_=biasd[:])

            # The LDWEIGHTS half of an f32r self-loading matmul accepts only
            # ONE sync wait, so real matmuls must never carry two. A tiny
            # bf16 ldweights "absorber" consumes each freshly-DMA'd tile's
            # semaphore (1 wait, no PSUM write, ~2 cycles); the real matmuls
            # then only ever wait on the PSUM-bank release from the
            # activation engine. The garbage weights it loads are irrelevant:
            # every f32r matmul self-loads its own weights.
            # absorb only the LEFT weight halves now; the right halves are
            # absorbed immediately before their first use (band 25) so the
            # in-order PE doesn't stall on the full 4.2MB weight load
            nc.tensor.ldweights(w_sb[0][0][:, 0:1].bitcast(bf16))
            nc.tensor.ldweights(w_sb[1][0][:, 0:1].bitcast(bf16))
            w_right_absorbed = False

            ci = 0
            ring = 0
            for i0 in range(0, K, 2):
                xt = xp.tile([P, 2, 2, T], fmm, tag="x")
                # input loads go through SWDGE (gpsimd) so their transfers
                # ride a separate DMA queue from the HWDGE output rings
                nc.gpsimd.dma_start(
                    out=xt[:],
                    in_=xT[i0 : i0 + 2].rearrange("b (kc p) t -> p b kc t", p=P),
                )
                nc.tensor.ldweights(xt[:, 0, 0, 0:1].bitcast(bf16))
                for i in (i0, i0 + 1):
                    while ci < NCHUNK and PLAN[ci]["band"] == i:
                        ch = PLAN[ci]
                        m, col0 = ch["M"], ch["col0"]
                        if col0 >= WSPLIT and not w_right_absorbed:
                            nc.tensor.ldweights(w_sb[0][1][:, 0:1].bitcast(bf16))
                            nc.tensor.ldweights(w_sb[1][1][:, 0:1].bitcast(bf16))
                            w_right_absorbed = True
                        ps = pp.tile([P, T], f32, tag="ps")
                        for kc in range(2):
                            if col0 < WSPLIT:
                                lhs = w_sb[kc][0][:, col0 : col0 + m]
                            else:
                                lhs = w_sb[kc][1][:, col0 - WSPLIT : col0 - WSPLIT + m]
                            nc.tensor.matmul(
                                ps[:m],
                                lhs,
                                xt[:, i - i0, kc],
                                start=(kc == 0),
                                stop=(kc == 1),
                            )
                        ot = op.tile([P, T], f32, tag="ot")
                        nc.scalar.activation(
                            ot[:m], ps[:m], tanh, bias=bias_sb[0:m, ci : ci + 1]
                        )
                        eng = nc.sync if ring == 0 else nc.scalar
                        ring ^= 1
                        eng.dma_start(
                            out=y[col0 : col0 + m, :], in_=ot[:m, :]
                        )
                        ci += 1
    nc.compile()
    return nc


def kernel(bands, Ws, bs):
    global LAST_RESULT
    from concourse.bass_utils import run_bass_kernel_spmd

    bands = np.asarray(bands, dtype=np.float32)
    Ws = [np.asarray(w, dtype=np.float32) for w in Ws]
    bs = [np.asarray(b, dtype=np.float32) for b in bs]

    xT = np.ascontiguousarray(bands.transpose(0, 1, 3, 2))  # (B, K, D, T)
    w_cat = np.ascontiguousarray(np.concatenate(Ws, axis=1))  # (D, OD_TOTAL)
    if MM_BF16:
        import ml_dtypes

        xT = xT.astype(ml_dtypes.bfloat16)
        w_cat = w_cat.astype(ml_dtypes.bfloat16)
    b_cat = np.concatenate(bs)
    bias_host = np.zeros((P, NCHUNK), dtype=np.float32)
    for ci, ch in enumerate(PLAN):
        bias_host[: ch["M"], ci] = b_cat[ch["col0"] : ch["col0"] + ch["M"]]

    nc = _build_bass()
    in_maps = [{"xT": xT[b], "w": w_cat, "bias": bias_host} for b in range(B)]
    res = run_bass_kernel_spmd(nc, in_maps, list(range(B)), trace=TRACE)
    LAST_RESULT = res

    out = np.zeros((B, C, N_FREQS, T), dtype=np.complex64)
    yall = np.stack([res.results[b]["y"] for b in range(B)])  # (B, 4096, T)
    g = yall[:, ROW_PERM, :].reshape(B, 2, C, 1024, T)
    out.real[:, :, :1024, :] = g[:, 0]
    out.imag[:, :, :1024, :] = g[:, 1]
    return out
